# revision 1
# baseline (speedup 1.0000x reference)
"""AddLinearAttention Trainium2 kernel — 8-core data-parallel over batch.

B=16, C=128, H=W=96. Each of 8 cores handles 2 batches, channel-major
(C=128 partitions, HW free). Depthwise convs run as PSUM-accumulating
diagonal matmuls over a zero-padded (100,100) image; kv via PE transposes.
"""

import os
from contextlib import ExitStack

import numpy as np

import concourse.bass as bass
import concourse.tile as tile
from concourse import bacc, mybir
from concourse._compat import with_exitstack
from concourse.bass_utils import run_bass_kernel_spmd

B, C, H, W = 16, 128, 96, 96
HW = H * W
NCORES = 8
BPC = B // NCORES          # batches per core
RPC = 4                    # image rows per chunk
NCH = H // RPC             # chunks per image (24)
PAD = 2
HP = H + 2 * PAD
WP = W + 2 * PAD
SCALE = C ** (-0.5)
S2 = SCALE / HW            # kv scale
ZS = SCALE / HW            # z scale folds k_mean's 1/HW and the C^-0.5

F32 = mybir.dt.float32
BF16 = mybir.dt.bfloat16
AF = mybir.ActivationFunctionType
ALU = mybir.AluOpType

# tap tables: (dy, dx) row-major
TAPS3 = [(dy, dx) for dy in (-1, 0, 1) for dx in (-1, 0, 1)]
TAPS5 = [(dy, dx) for dy in (-2, -1, 0, 1, 2) for dx in (-2, -1, 0, 1, 2)]
NDIAG = 3 * 9 + 25  # 52


@with_exitstack
def _build(ctx: ExitStack, tc: tile.TileContext):
    nc = tc.nc
    x_d = nc.dram_tensor("x", [BPC, C, H, W], F32, kind="ExternalInput").ap()
    wqkvo_d = nc.dram_tensor("wqkvo_t", [C, 4 * C], F32, kind="ExternalInput").ap()
    wproj_d = nc.dram_tensor("wproj_t", [C, C], F32, kind="ExternalInput").ap()
    diag_d = nc.dram_tensor("dwdiag", [C, NDIAG, C], F32, kind="ExternalInput").ap()
    bias_d = nc.dram_tensor("biases", [C, 9], F32, kind="ExternalInput").ap()
    dwcol_d = nc.dram_tensor("dwcols", [C, NDIAG], F32, kind="ExternalInput").ap()
    ident_d = nc.dram_tensor("ident", [C, C], F32, kind="ExternalInput").ap()
    out_d = nc.dram_tensor("out", [BPC, C, H, W], F32, kind="ExternalOutput").ap()

    const = ctx.enter_context(tc.tile_pool(name="const", bufs=1))
    wq_sb = const.tile([C, 4 * C], BF16, tag="wq")
    wp_sb = const.tile([C, C], BF16, tag="wp")
    dg_sb = const.tile([C, NDIAG, C], BF16, tag="dg")
    bi_sb = const.tile([C, 9], F32, tag="bi")
    id_sb = const.tile([C, C], BF16, tag="id")
    ones_sb = const.tile([C, C], BF16, tag="ones")
    inv_sb = const.tile([C, C], BF16, tag="inv")
    kmr_sb = const.tile([C, C], BF16, tag="kmr")
    ks_sb = const.tile([C, 1], F32, tag="ks")
    dwc_sb = const.tile([C, NDIAG], F32, tag="dwc")

    # f32 -> bf16 casts ride the SWDGE DMA
    nc.gpsimd.dma_start(out=wq_sb[:], in_=wqkvo_d[:])
    nc.gpsimd.dma_start(out=wp_sb[:], in_=wproj_d[:])
    nc.gpsimd.dma_start(out=dg_sb[:], in_=diag_d[:])
    nc.sync.dma_start(out=bi_sb[:], in_=bias_d[:])
    nc.sync.dma_start(out=dwc_sb[:], in_=dwcol_d[:])
    nc.gpsimd.dma_start(out=id_sb[:], in_=ident_d[:])
    zer_sb = const.tile([C, RPC, W], BF16, tag="zer")
    nc.vector.memset(zer_sb[:], 0.0)
    nc.vector.memset(ones_sb[:], 1.0)
    nc.vector.memset(inv_sb[:], 1.0 / C)

    bq, bk, bv, bo = (bi_sb[:, i : i + 1] for i in range(4))
    bcq, bck, bcv, blep, bprj = (bi_sb[:, i : i + 1] for i in range(4, 9))

    pads = ctx.enter_context(tc.tile_pool(name="pads", bufs=1))
    qpad = pads.tile([C, HP, WP], BF16, tag="qpad")
    kpad = pads.tile([C, HP, WP], BF16, tag="kpad")
    vpad = pads.tile([C, HP, WP], BF16, tag="vpad")
    for t in (qpad, kpad, vpad):
        nc.vector.memset(t[:], 0.0)

    flats = ctx.enter_context(tc.tile_pool(name="flats", bufs=1))

    xin = ctx.enter_context(tc.tile_pool(name="xin", bufs=1))
    ttmp = ctx.enter_context(tc.tile_pool(name="ttmp", bufs=4))
    etmp = ctx.enter_context(tc.tile_pool(name="etmp", bufs=2))
    echn = ctx.enter_context(tc.tile_pool(name="echn", bufs=1))
    ostg = ctx.enter_context(tc.tile_pool(name="ostg", bufs=2))
    kvs = ctx.enter_context(tc.tile_pool(name="kvs", bufs=1))
    kv_sb = kvs.tile([C, C], BF16, tag="kv")

    for b in range(BPC):
        x_sb = xin.tile([C, H, W], BF16, tag="x")
        o_sb = flats.tile([C, H, W], BF16, tag="o")
        qh = flats.tile([C, H, W], BF16, tag="qh")     # elu(q+cq)+1
        kh = flats.tile([C, H, W], BF16, tag="kh")
        vh = flats.tile([C, H, W], BF16, tag="vh")
        lep = flats.tile([C, H, W], BF16, tag="lep")
        res = flats.tile([C, H, W], BF16, tag="kh")    # kh dead after kv phase
        nc.gpsimd.dma_start(out=x_sb[:], in_=x_d[b])

        # ---- fused: qkvo 1x1 conv (leads by 1 chunk) + depthwise convs
        with tc.tile_pool(name="ps_a", bufs=2, space="PSUM") as ps, \
             tc.tile_pool(name="cps2", bufs=2, space="PSUM") as cps2, \
             tc.tile_pool(name="cps", bufs=1, space="PSUM") as cps:
          for rr in range(NCH + 1):
            if rr < NCH:
                y0 = rr * RPC
                rx = x_sb[:, y0 : y0 + RPC, :]
                for g, (dst, bb) in enumerate(
                    ((qpad, bq), (kpad, bk), (vpad, bv), (o_sb, bo))
                ):
                    p = ps.tile([C, RPC, W], F32, tag="qkvo")
                    nc.tensor.matmul(
                        p[:], wq_sb[:, g * C : (g + 1) * C], rx, start=True, stop=True
                    )
                    if g < 3:
                        dstw = dst[:, PAD + y0 : PAD + y0 + RPC, PAD : PAD + W]
                    else:
                        dstw = dst[:, y0 : y0 + RPC, :]
                    if g % 2 == 0:
                        nc.vector.tensor_scalar_add(dstw, p[:], bb)
                    else:
                        nc.scalar.activation(dstw, p[:], AF.Identity, bias=bb)
            if rr < 1:
                continue
            r = rr - 1
            y0 = r * RPC

            def win(src, dy, dx):
                return src[
                    :,
                    PAD + y0 + dy : PAD + y0 + dy + RPC,
                    PAD + dx : PAD + dx + W,
                ]

            def dwconv(src, taps, t0, tag, skip=()):
                pool = cps2 if tag in ("cq", "ck") else cps
                p = pool.tile([C, RPC, W], F32, tag=tag)
                pe_taps = [(i, t) for i, t in enumerate(taps) if t not in skip]
                for j, (i, (dy, dx)) in enumerate(pe_taps):
                    nc.tensor.matmul(
                        p[:],
                        dg_sb[:, t0 + i, :],
                        win(src, dy, dx),
                        start=(j == 0),
                        stop=(j == len(pe_taps) - 1),
                    )
                return p

            OT = (0, 1)                    # tap offloaded from PE (3x3 convs)
            OTI = TAPS3.index(OT)
            cq = dwconv(qpad, TAPS3, 0, "cq", skip=(OT,))
            ck = dwconv(kpad, TAPS3, 9, "ck", skip=(OT,))
            cv = dwconv(vpad, TAPS3, 18, "cv", skip=(OT,))
            LSKIP = ((0, 1), (0, 2), (0, -1), (0, -2))
            cl = dwconv(vpad, TAPS5, 27, "cl", skip=LSKIP)

            for src, cp, bb, dst, t0, eng in (
                (qpad, cq, bcq, qh, 0, nc.vector),
                (kpad, ck, bck, kh, 9, nc.vector),
            ):
                w0 = src[:, PAD + y0 : PAD + y0 + RPC, PAD : PAD + W]
                c1 = etmp.tile([C, RPC, W], BF16, tag="c1")
                eng.scalar_tensor_tensor(
                    c1[:], win(src, *OT), dwc_sb[:, t0 + OTI : t0 + OTI + 1],
                    w0, ALU.mult, ALU.add,
                )
                s = etmp.tile([C, RPC, W], BF16, tag="s")
                nc.vector.scalar_tensor_tensor(s[:], cp[:], bb, c1[:], ALU.add, ALU.add)
                e = etmp.tile([C, RPC, W], BF16, tag="e")
                nc.scalar.activation(e[:], s[:], AF.Exp)
                rl = etmp.tile([C, RPC, W], BF16, tag="rl")
                nc.vector.tensor_scalar_max(rl[:], s[:], 0.0)
                nc.vector.scalar_tensor_tensor(
                    dst[:, y0 : y0 + RPC, :], e[:], 1.0, rl[:], ALU.min, ALU.add
                )
            vwin = vpad[:, PAD + y0 : PAD + y0 + RPC, PAD : PAD + W]
            cv1 = etmp.tile([C, RPC, W], BF16, tag="cv1")
            nc.vector.scalar_tensor_tensor(
                cv1[:], win(vpad, *OT), dwc_sb[:, 18 + OTI : 18 + OTI + 1],
                vwin, ALU.mult, ALU.add,
            )
            nc.vector.scalar_tensor_tensor(
                vh[:, y0 : y0 + RPC, :], cv[:], bcv, cv1[:], ALU.add, ALU.add
            )
            lch = None
            for li, lt in enumerate(LSKIP):
                ti = 27 + TAPS5.index(lt)
                lnew = etmp.tile([C, RPC, W], BF16, tag=f"lc{li % 2}")
                eng = nc.vector
                if lch is None:
                    eng.tensor_scalar(
                        lnew[:], win(vpad, *lt), dwc_sb[:, ti : ti + 1], None, ALU.mult
                    )
                else:
                    eng.scalar_tensor_tensor(
                        lnew[:], win(vpad, *lt), dwc_sb[:, ti : ti + 1],
                        lch[:], ALU.mult, ALU.add,
                    )
                lch = lnew
            nc.vector.scalar_tensor_tensor(
                lep[:, y0 : y0 + RPC, :], cl[:], blep, lch[:], ALU.add, ALU.add
            )

        # ---- k_mean -> replicated lhsT (scaled)
        nc.vector.tensor_reduce(ks_sb[:], kh[:], axis=mybir.AxisListType.XY, op=ALU.add)
        nc.vector.tensor_scalar(
            kmr_sb[:], ones_sb[:], ks_sb[:], ZS, ALU.mult, ALU.mult
        )

        # ---- kv = s2 * k~^T v~ via PE transposes, 72 token chunks
        with tc.tile_pool(name="kvp", bufs=1, space="PSUM") as kvp, \
             tc.tile_pool(name="tpp", bufs=3, space="PSUM") as tpp, \
             tc.tile_pool(name="tpp2", bufs=4, space="PSUM") as tpp2:
          kvacc = kvp.tile([C, C], F32, tag="kvacc")
          kf = kh[:].rearrange("p a b -> p (a b)")
          vf = vh[:].rearrange("p a b -> p (a b)")
          for j in range(HW // C):
            tp1 = tpp.tile([C, C], BF16, tag="tp1")
            nc.tensor.transpose(tp1[:], kf[:, j * C : (j + 1) * C], id_sb[:])
            kt = ttmp.tile([C, C], BF16, tag="kt")
            nc.vector.tensor_copy(kt[:], tp1[:])
            tp2 = tpp2.tile([C, C], BF16, tag="tp2")
            nc.tensor.transpose(tp2[:], vf[:, j * C : (j + 1) * C], id_sb[:])
            vt = ttmp.tile([C, C], BF16, tag="vt")
            nc.scalar.activation(vt[:], tp2[:], AF.Copy)
            nc.tensor.matmul(
                kvacc[:], kt[:], vt[:], start=(j == 0), stop=(j == HW // C - 1)
            )
          nc.scalar.activation(kv_sb[:], kvacc[:], AF.Copy, scale=float(S2))

        # ---- res = (q~ @ kv)(1 + 1/z) - z*vbar ; + lepe ; * o ; proj ; out
        with tc.tile_pool(name="ps_e", bufs=2, space="PSUM") as ps, \
             tc.tile_pool(name="ps_f", bufs=2, space="PSUM") as psf:
          for r in range(NCH):
            y0 = r * RPC
            rq = qh[:, y0 : y0 + RPC, :]
            rv = vh[:, y0 : y0 + RPC, :]
            zp = ps.tile([C, RPC, W], F32, tag="zp")
            nc.tensor.matmul(zp[:], kmr_sb[:], rq, start=True, stop=True)
            vb = ps.tile([C, RPC, W], F32, tag="vb")
            nc.tensor.matmul(vb[:], inv_sb[:], rv, start=True, stop=True)
            rp = ps.tile([C, RPC, W], F32, tag="rp")
            nc.tensor.matmul(rp[:], kv_sb[:], rq, start=True, stop=True)

            rr = etmp.tile([C, RPC, W], F32, tag="rr")
            nc.vector.reciprocal(rr[:], zp[:])
            nc.vector.tensor_scalar_add(rr[:], rr[:], 1.0)
            t1 = echn.tile([C, RPC, W], BF16, tag="t1")
            nc.vector.tensor_tensor(t1[:], rp[:], rr[:], ALU.mult)
            vbs = etmp.tile([C, RPC, W], BF16, tag="vbs")
            nc.scalar.activation(vbs[:], vb[:], AF.Copy)
            t2 = echn.tile([C, RPC, W], BF16, tag="t2")
            nc.vector.tensor_tensor(t2[:], zp[:], vbs[:], ALU.mult)
            t3 = echn.tile([C, RPC, W], BF16, tag="t3")
            nc.gpsimd.tensor_tensor(t3[:], t1[:], t2[:], ALU.subtract)
            t4 = echn.tile([C, RPC, W], BF16, tag="t4")
            nc.gpsimd.tensor_tensor(t4[:], t3[:], lep[:, y0 : y0 + RPC, :], ALU.add)
            nc.gpsimd.tensor_tensor(
                res[:, y0 : y0 + RPC, :], t4[:], o_sb[:, y0 : y0 + RPC, :], ALU.mult
            )
            pp = psf.tile([C, RPC, W], F32, tag="pp")
            nc.tensor.matmul(
                pp[:], wp_sb[:], res[:, y0 : y0 + RPC, :], start=True, stop=True
            )
            og = ostg.tile([C, RPC, W], F32, tag="og")
            nc.scalar.activation(og[:], pp[:], AF.Copy)
            nc.vector.tensor_scalar_add(og[:], og[:], bprj)
            nc.sync.dma_start(out=out_d[b, :, y0 : y0 + RPC, :], in_=og[:])


_CACHE = {}


def _get_nc():
    if "nc" not in _CACHE:
        nc = bacc.Bacc("TRN2", target_bir_lowering=False, debug=False)
        with tile.TileContext(nc, pool_alloc_mode="queue") as tc:
            _build(tc)
        nc.compile()
        _CACHE["nc"] = nc
    return _CACHE["nc"]


def kernel(**inputs) -> np.ndarray:
    x = np.asarray(inputs["x"], np.float32)
    w_qkvo = np.asarray(inputs["w_qkvo"], np.float32)[:, :, 0, 0]  # (4C, C)
    b_qkvo = np.asarray(inputs["b_qkvo"], np.float32)
    w_lepe = np.asarray(inputs["w_lepe"], np.float32)[:, 0]        # (C,5,5)
    b_lepe = np.asarray(inputs["b_lepe"], np.float32)
    w_proj = np.asarray(inputs["w_proj"], np.float32)[:, :, 0, 0]
    b_proj = np.asarray(inputs["b_proj"], np.float32)
    w_q = np.asarray(inputs["w_q"], np.float32)[:, 0]              # (C,3,3)
    b_q = np.asarray(inputs["b_q"], np.float32)
    w_k = np.asarray(inputs["w_k"], np.float32)[:, 0]
    b_k = np.asarray(inputs["b_k"], np.float32)
    w_v = np.asarray(inputs["w_v"], np.float32)[:, 0]
    b_v = np.asarray(inputs["b_v"], np.float32)

    diags = np.zeros((NDIAG, C, C), np.float32)
    idx = 0
    for wt, taps in ((w_q, TAPS3), (w_k, TAPS3), (w_v, TAPS3), (w_lepe, TAPS5)):
        k = int(np.sqrt(len(taps)))
        for ky in range(k):
            for kx in range(k):
                np.fill_diagonal(diags[idx], wt[:, ky, kx])
                idx += 1
    dwdiag = np.ascontiguousarray(diags.transpose(1, 0, 2))  # (C, 52, C)

    biases = np.stack(
        [
            b_qkvo[:C], b_qkvo[C : 2 * C], b_qkvo[2 * C : 3 * C], b_qkvo[3 * C :],
            b_q, b_k, b_v, b_lepe, b_proj,
        ],
        axis=1,
    ).astype(np.float32)

    dwcols = np.zeros((NDIAG, C), np.float32)
    idx = 0
    for wt, taps in ((w_q, TAPS3), (w_k, TAPS3), (w_v, TAPS3), (w_lepe, TAPS5)):
        k = int(np.sqrt(len(taps)))
        for ky in range(k):
            for kx in range(k):
                dwcols[idx] = wt[:, ky, kx]
                idx += 1

    shared = {
        "dwcols": np.ascontiguousarray(dwcols.T),
        "wqkvo_t": np.ascontiguousarray(w_qkvo.T),
        "wproj_t": np.ascontiguousarray(w_proj.T),
        "dwdiag": dwdiag,
        "biases": biases,
        "ident": np.eye(C, dtype=np.float32),
    }
    xb = x.reshape(NCORES, BPC, C, H, W)
    in_maps = [{"x": np.ascontiguousarray(xb[i]), **shared} for i in range(NCORES)]

    nc = _get_nc()
    _CACHE["last_in_maps"] = in_maps
    r = run_bass_kernel_spmd(
        nc, in_maps, core_ids=list(range(NCORES)),
        trace=bool(int(os.environ.get("KERNEL_TRACE", "0"))),
    )
    _CACHE["last_results"] = r
    out = np.stack([r.results[i]["out"] for i in range(NCORES)])  # (8,2,C,H,W)
    return out.reshape(B, C, H, W)

